# revision 65
# baseline (speedup 1.0000x reference)
"""Trainium2 Bass kernel for nn_MultiHeadAttention (B=2, S=2048, d_model=1024, H=16).

Sharding (8 cores): data-parallel over B (2) x tensor-parallel over head groups
(4 groups of 4 heads).  Each core computes its head-group's Q/K/V projections
(column-sharded weights), attention for its 4 heads, and a row-parallel
out_proj partial product.  The host sums the 4 partials per batch (the
"all-reduce") and adds the output bias.

All on-chip layouts are transposed ([feature, seq]) so that:
  - scores are computed directly transposed  S_T[k,q] = Kh @ Qh^T  (no P
    transpose needed before P@V),
  - softmax denominators come from ones-vector matmuls (col-tiled 4-way),
  - the PE array is fully packed for dk=64 heads via row/col tile_position
    pairing (auto-derived from AP base partitions),
  - the k-loop is software-pipelined one stage (scores of tile k overlap
    exp and P@V of tile k-1), inputs stream on both HWDGE queues.

Dtypes: inputs/projections and the P,V operands are fp16 (1 cyc/row on
the PE, fp32 PSUM accumulation everywhere); scores and out_proj operands
are float32r (TF32 path, 1 cyc/row at N>=256; note f32r cannot be
col-tiled -- XBUS budget -- which is why the P@V/sums side is fp16).
"""

import sys
import numpy as np

for _p in ("/opt/trn_rl_repo", "/root/.axon_site/_ro/trn_rl_repo"):
    if _p not in sys.path:
        sys.path.append(_p)

D_MODEL = 1024
NUM_HEADS = 16
DK = 64
B = 2
S = 2048
N_CORES = 8
HPC = 4               # heads per core
E = HPC * DK          # 256 features per core
NQ = 512              # q-chunk size
N_QC = S // NQ        # 4 q chunks
N_KT = S // 128       # 16 k tiles
N_DT = D_MODEL // 128  # 8 contraction tiles for projections

_PROGRAM = None
_RUN_KWARGS = {}      # test harness may set {"trace": True}
_LAST_RESULTS = None  # BassKernelResults of the last run


def _build_program():
    import concourse.bass as bass
    import concourse.mybir as mybir
    from concourse import bacc, tile
    from contextlib import ExitStack

    f32 = mybir.dt.float32
    f32r = mybir.dt.float32r
    bf16 = mybir.dt.bfloat16
    fp16 = mybir.dt.float16
    AF = mybir.ActivationFunctionType

    nc = bacc.Bacc("TRN2", target_bir_lowering=False, debug=False,
                   num_devices=N_CORES)

    # Per-core DRAM I/O (transposed activations, pre-sliced weights)
    qT = nc.dram_tensor("qT", [D_MODEL, S], mybir.dt.float16, kind="ExternalInput").ap()
    kT = nc.dram_tensor("kT", [D_MODEL, S], mybir.dt.float16, kind="ExternalInput").ap()
    vT = nc.dram_tensor("vT", [D_MODEL, S], mybir.dt.float16, kind="ExternalInput").ap()
    wq = nc.dram_tensor("wq", [D_MODEL, E], mybir.dt.float16, kind="ExternalInput").ap()
    wk = nc.dram_tensor("wk", [D_MODEL, E], mybir.dt.float16, kind="ExternalInput").ap()
    wv = nc.dram_tensor("wv", [D_MODEL, E], mybir.dt.float16, kind="ExternalInput").ap()
    wo = nc.dram_tensor("wo", [E, D_MODEL], f32r, kind="ExternalInput").ap()
    bq = nc.dram_tensor("bq", [E, 1], f32, kind="ExternalInput").ap()
    bk = nc.dram_tensor("bk", [E, 1], f32, kind="ExternalInput").ap()
    bv = nc.dram_tensor("bv", [E, 1], f32, kind="ExternalInput").ap()
    onesl = nc.dram_tensor("onesl", [1, 64], f32r, kind="ExternalInput").ap()
    onesk = nc.dram_tensor("onesk", [128, 1], mybir.dt.float16,
                           kind="ExternalInput").ap()
    zT = nc.dram_tensor("zT", [D_MODEL, S], f32, kind="ExternalOutput").ap()

    def r(ap):  # operands are natively f32r now
        return ap

    with tile.TileContext(nc) as tc, ExitStack() as ctx:
        persist = ctx.enter_context(tc.tile_pool(name="persist", bufs=1))
        const = ctx.enter_context(tc.tile_pool(name="const", bufs=1))

        # Weights resident in SBUF: [128, n_dt, E]-style views
        wq_sb = persist.tile([128, N_DT, E], fp16, tag="wq", name="wq")
        wk_sb = persist.tile([128, N_DT, E], fp16, tag="wk", name="wk")
        wv_sb = persist.tile([128, N_DT, E], fp16, tag="wv", name="wv")
        wo_sb = persist.tile([128, 2, D_MODEL], f32r, tag="wo", name="wo")
        # wk/wq first (gate the K/Q projections), split across queues;
        # wv/wo stream later behind the K inputs
        nc.sync.dma_start(wk_sb[:], wk.rearrange("(t p) e -> p t e", p=128))
        nc.scalar.dma_start(wq_sb[:], wq.rearrange("(t p) e -> p t e", p=128))
        nc.scalar.dma_start(wv_sb[:], wv.rearrange("(t p) e -> p t e", p=128))
        nc.sync.dma_start(wo_sb[:], wo.rearrange("(t p) e -> p t e", p=128))

        bq_sb = persist.tile([128, 2], f32, tag="bq", name="bq")
        bk_sb = persist.tile([128, 2], f32, tag="bk", name="bk")
        bv_sb = persist.tile([128, 2], f32, tag="bv", name="bv")
        nc.sync.dma_start(bq_sb[:], bq.rearrange("(m p) o -> p (m o)", p=128))
        nc.sync.dma_start(bk_sb[:], bk.rearrange("(m p) o -> p (m o)", p=128))
        nc.sync.dma_start(bv_sb[:], bv.rearrange("(m p) o -> p (m o)", p=128))

        from concourse.masks import make_identity
        ident = const.tile([128, 128], fp16, tag="ident", name="ident")
        make_identity(nc, ident)
        # host-provided constants: ones column (sums lhsT) and the
        # pair-broadcast selector
        ones_k = const.tile([128, 1], fp16, tag="ones_k", name="ones_k")
        ones_l = const.tile([1, 64], f32r, tag="ones_l", name="ones_l")
        nc.sync.dma_start(ones_k[:], onesk)
        nc.sync.dma_start(ones_l[:], onesl)

        # Projection outputs (transposed): pair tensors hold 2 heads each
        qh = [persist.tile([128, S], f32r, tag=f"qh{p}", name=f"qh{p}") for p in range(2)]
        kh = [persist.tile([128, S], f32r, tag=f"kh{p}", name=f"kh{p}") for p in range(2)]
        # Vh non-transposed [k, e], s-tile-major columns
        vh = persist.tile([128, N_KT * E], fp16, tag="vh", name="vh")
        # normalized attention output (transposed), pair tensors
        ot = [persist.tile([128, S], f32r, tag=f"ot{p}", name=f"ot{p}") for p in range(2)]

        stage_a = ExitStack()
        xpool = stage_a.enter_context(tc.tile_pool(name="xpool", bufs=8))
        apsum = stage_a.enter_context(
            tc.tile_pool(name="apsum", bufs=8, space="PSUM"))

        # vhT: transposed V projection [e, s] (bf16), transposed to vh after
        vhT = [persist.tile([128, S], fp16, tag=f"vhT{m}", name=f"vhT{m}")
               for m in range(2)]

        # ---- Stage A: projections (all transposed orientation) ---------
        dma_engines = (nc.sync, nc.scalar)  # two HWDGE queues
        for which, xdram, w_sb, b_sb, dst in (
            ("k", kT, wk_sb, bk_sb, kh),
            ("q", qT, wq_sb, bq_sb, qh),
            ("v", vT, wv_sb, bv_sb, vhT),
        ):
            # ps[m][n]: out rows m*128, cols n*512
            ps = [[apsum.tile([128, 512], f32, tag="aps", name="aps") for n in range(4)]
                  for m in range(2)]
            for d in range(N_DT):
                xt = xpool.tile([128, S], fp16, tag="xt", name="xt")
                dma_engines[d % 2].dma_start(xt[:], xdram[d * 128:(d + 1) * 128, :])
                for m in range(2):
                    lhsT = w_sb[:, d, m * 128:(m + 1) * 128]
                    for n in range(4):
                        nc.tensor.matmul(
                            ps[m][n][:], r(lhsT), r(xt[:, n * 512:(n + 1) * 512]),
                            start=(d == 0), stop=(d == N_DT - 1))
            for m in range(2):
                for n in range(4):
                    nc.vector.tensor_scalar_add(
                        dst[m][:, n * 512:(n + 1) * 512], ps[m][n][:],
                        b_sb[:, m:m + 1])

        # vh[s, e] = vhT^T via PE transposes (4 blocks per psum bank)
        for st in range(N_KT):
            tp = apsum.tile([128, 512], fp16, tag="aps", name="tps") \
                if st % 2 == 0 else tp
            for m in range(2):
                j = (st % 2) * 2 + m
                nc.tensor.matmul(
                    tp[:, j * 128:(j + 1) * 128],
                    vhT[m][:, st * 128:(st + 1) * 128], ident[:],
                    is_transpose=True, start=True, stop=True,
                    skip_group_check=True)
                nc.vector.tensor_copy(
                    vh[:, st * E + m * 128: st * E + (m + 1) * 128],
                    tp[:, j * 128:(j + 1) * 128])

        stage_a.close()

        # ---- Stage B: attention + out_proj, per q-chunk ----------------
        scp = ctx.enter_context(tc.tile_pool(name="scp", bufs=2, space="PSUM"))
        outp = ctx.enter_context(tc.tile_pool(name="outp", bufs=2, space="PSUM"))
        sump = ctx.enter_context(tc.tile_pool(name="sump", bufs=1, space="PSUM"))
        zp = ctx.enter_context(tc.tile_pool(name="zp", bufs=1, space="PSUM"))

        ptp = ctx.enter_context(tc.tile_pool(name="ptp", bufs=8))
        rp = ctx.enter_context(tc.tile_pool(name="rp", bufs=6))
        bcp = ctx.enter_context(tc.tile_pool(name="bcp", bufs=3))
        zsb = ctx.enter_context(tc.tile_pool(name="zsb", bufs=4))

        for qc in range(N_QC):
            q0, q1 = qc * NQ, (qc + 1) * NQ
            outs = [outp.tile([128, NQ], f32, tag="outp", name="outp") for _ in range(2)]
            sums = sump.tile([128, NQ], f32, tag="sums", name="sums")

            def pv_sums(kt, pts):
                # P@V + denominator for k-tile kt (pts = pair pt tiles)
                for p in range(2):
                    for j in range(2):
                        h = 2 * p + j
                        lo, hi = j * 64, (j + 1) * 64
                        ptj = pts[p][:, j * NQ:(j + 1) * NQ]
                        # P@V (col-tiled pair: head j -> out partitions j*64)
                        nc.tensor.matmul(
                            outs[p][lo:hi, :],
                            r(vh[:, kt * E + h * 64: kt * E + (h + 1) * 64]),
                            r(ptj), start=(kt == 0), stop=(kt == N_KT - 1),
                            skip_group_check=True)
                        # softmax denominator (col-tiled 4-way, M=1)
                        nc.tensor.matmul(
                            sums[32 * h:32 * h + 1, :], r(ones_k[:]), r(ptj),
                            start=(kt == 0), stop=(kt == N_KT - 1),
                            tile_position=(0, 32 * h), skip_group_check=True)

            # k-loop software-pipelined one stage deep: scores(kt) issue on
            # PE while exp(kt-1) runs on ACT and pv/sums(kt-1) follows.
            prev_pts = None
            for kt in range(N_KT):
                k0 = kt * 128
                scs = []
                for p in range(2):
                    # both heads' scores side by side in one 2-bank psum tile
                    sc = scp.tile([128, 2 * NQ], f32, tag="sc", name="sc")
                    for j in range(2):
                        lo, hi = j * 64, (j + 1) * 64
                        nc.tensor.matmul(
                            sc[:, j * NQ:(j + 1) * NQ],
                            r(kh[p][lo:hi, k0:k0 + 128]),
                            r(qh[p][lo:hi, q0:q1]), start=True, stop=True)
                    scs.append(sc)
                if prev_pts is not None:
                    pv_sums(kt - 1, prev_pts)
                pts = []
                for p in range(2):
                    # one wide exp per pair (amortizes ACT fixed cost)
                    pt = ptp.tile([128, 2 * NQ], fp16, tag="pt", name="pt")
                    nc.scalar.activation(pt[:], scs[p][:], AF.Exp, scale=0.125)
                    pts.append(pt)
                prev_pts = pts
            pv_sums(N_KT - 1, prev_pts)
            # normalize: ot = outs * (1/sums) broadcast across partitions
            for p in range(2):
                bc_sb = bcp.tile([128, NQ], f32, tag="bc_sb", name="bc_sb")
                for j in range(2):
                    h = 2 * p + j
                    rv = rp.tile([1, NQ], f32r, tag="rv", name="rv")
                    with nc.allow_low_precision(reason="tf32 softmax recip"):
                        nc.vector.reciprocal(rv[:], sums[32 * h:32 * h + 1, :])
                    # rank-1 broadcast of 1/sum across 64 partitions (PE);
                    # separate base-0 psum tile (f32r can't col-tile)
                    bc = scp.tile([64, NQ], f32, tag="sc", name="bcps")
                    nc.tensor.matmul(bc[:], ones_l[:], rv[:],
                                     start=True, stop=True)
                    nc.vector.tensor_copy(bc_sb[j * 64:(j + 1) * 64, :], bc[:])
                nc.vector.tensor_mul(ot[p][:, q0:q1], outs[p][:], bc_sb[:])
            # out_proj partial: zT[e, q-chunk]
            for e in range(8):
                pool_, tag_ = (zp, "zps") if e % 2 == 0 else (sump, "sums")
                zps = pool_.tile([128, NQ], f32, tag=tag_, name="zps")
                for c in range(2):
                    nc.tensor.matmul(
                        zps[:], r(wo_sb[:, c, e * 128:(e + 1) * 128]),
                        r(ot[c][:, q0:q1]), start=(c == 0), stop=(c == 1))
                zt_sb = zsb.tile([128, NQ], f32, tag="zt_sb", name="zt_sb")
                nc.vector.tensor_copy(zt_sb[:], zps[:])
                dma_engines[e % 2].dma_start(
                    zT[e * 128:(e + 1) * 128, q0:q1], zt_sb[:])

    nc.compile()
    return nc


def _get_program():
    global _PROGRAM
    if _PROGRAM is None:
        _PROGRAM = _build_program()
    return _PROGRAM


ONESL_NP = None
ONESK_NP = None


def _init_consts():
    global ONESL_NP, ONESK_NP
    if ONESL_NP is None:
        import ml_dtypes
        ONESL_NP = np.ones((1, 64), dtype=np.float32)
        ONESK_NP = np.ones((128, 1), np.float16)


def _make_in_maps(q, k, v, Wq, bq, Wk, bk, Wv, bv, Wo):
    _init_consts()
    f32 = np.float32
    xT = {}
    for b in range(B):
        xT[("q", b)] = np.ascontiguousarray(q[b].T, dtype=np.float16)
        xT[("k", b)] = np.ascontiguousarray(k[b].T, dtype=np.float16)
        xT[("v", b)] = np.ascontiguousarray(v[b].T, dtype=np.float16)
    wslices = {}
    for g in range(4):
        sl = slice(g * E, (g + 1) * E)
        wslices[("wq", g)] = np.ascontiguousarray(Wq[sl, :].T, dtype=np.float16)
        wslices[("wk", g)] = np.ascontiguousarray(Wk[sl, :].T, dtype=np.float16)
        wslices[("wv", g)] = np.ascontiguousarray(Wv[sl, :].T, dtype=np.float16)
        wslices[("wo", g)] = np.ascontiguousarray(Wo[:, sl].T, dtype=f32)
        wslices[("bq", g)] = np.ascontiguousarray(bq[sl].reshape(E, 1), dtype=f32)
        wslices[("bk", g)] = np.ascontiguousarray(bk[sl].reshape(E, 1), dtype=f32)
        wslices[("bv", g)] = np.ascontiguousarray(bv[sl].reshape(E, 1),
                                                   dtype=f32)
    in_maps = []
    for c in range(N_CORES):
        b, g = c // 4, c % 4
        in_maps.append({
            "onesl": ONESL_NP, "onesk": ONESK_NP,
            "qT": xT[("q", b)], "kT": xT[("k", b)], "vT": xT[("v", b)],
            "wq": wslices[("wq", g)], "wk": wslices[("wk", g)],
            "wv": wslices[("wv", g)], "wo": wslices[("wo", g)],
            "bq": wslices[("bq", g)], "bk": wslices[("bk", g)],
            "bv": wslices[("bv", g)],
        })
    return in_maps


def _numpy_fallback(q, k, v, mask, Wq, bq, Wk, bk, Wv, bv, Wo, bo):
    # Only used if mask is not all-True (never the case for this problem).
    def proj(x, W, b_):
        y = x @ W.T + b_
        return y.reshape(B, S, NUM_HEADS, DK).transpose(0, 2, 1, 3)
    qh, kh, vh = proj(q, Wq, bq), proj(k, Wk, bk), proj(v, Wv, bv)
    sc = np.einsum("bhqd,bhkd->bhqk", qh, kh) / np.sqrt(DK)
    sc = np.where(mask, sc, np.float32(-1e9))
    sc = sc - sc.max(-1, keepdims=True)
    p = np.exp(sc)
    p /= p.sum(-1, keepdims=True)
    o = np.einsum("bhqk,bhkd->bhqd", p, vh)
    o = o.transpose(0, 2, 1, 3).reshape(B, S, D_MODEL)
    return (o @ Wo.T + bo).astype(np.float32)


def kernel(q, k, v, mask, Wq, bq, Wk, bk, Wv, bv, Wo, bo):
    q = np.asarray(q, dtype=np.float32)
    k = np.asarray(k, dtype=np.float32)
    v = np.asarray(v, dtype=np.float32)
    Wq, Wk, Wv, Wo = (np.asarray(w, dtype=np.float32) for w in (Wq, Wk, Wv, Wo))
    bq, bk, bv, bo = (np.asarray(x, dtype=np.float32) for x in (bq, bk, bv, bo))
    if not np.all(np.asarray(mask)):
        return _numpy_fallback(q, k, v, np.asarray(mask), Wq, bq, Wk, bk,
                               Wv, bv, Wo, bo)

    from concourse.bass_utils import run_bass_kernel_spmd
    nc = _get_program()
    in_maps = _make_in_maps(q, k, v, Wq, bq, Wk, bk, Wv, bv, Wo)
    res = run_bass_kernel_spmd(nc, in_maps, core_ids=list(range(N_CORES)),
                               **_RUN_KWARGS)
    global _LAST_RESULTS
    _LAST_RESULTS = res
    out = np.empty((B, S, D_MODEL), dtype=np.float32)
    for b in range(B):
        acc = res.results[4 * b]["zT"].astype(np.float32).copy()
        for g in range(1, 4):
            acc += res.results[4 * b + g]["zT"]
        out[b] = acc.T + bo
    return out
